# revision 15
# baseline (speedup 1.0000x reference)
"""Trainium2 Bass kernel for nn_FSMNSeleNetV3 (FSMN stack + channel maxpool + decoder).

Self-contained: hardcodes all shapes from the problem spec and only imports
numpy + the concourse stack from /opt/trn_rl_repo.

Sharding: pure data parallel over batch. Each of the 8 cores processes 4
batches x 4 channels = 16 independent sequences of T=2048 tokens.

v3 design (layer-major, multi-engine balanced, PE tile-concurrency):
- All activations bf16 in SBUF; matmuls bf16 (1 cycle/row on the PE).
- 64-channel tensors (h, o) pack the two T/2 time-halves onto 128 partitions.
- Layer-major emission: each unit stage streams all 16 sequences so every
  engine always has independent work and the PE never idles (HAM stays warm).
- Expands (K=64) emit as adjacent row-group tile pairs (tile_position (0,0)
  and (64,0)); shrinks (M=64) as adjacent col-group pairs (out partition 0/64)
  - disjoint PE sub-arrays run these pairs concurrently on hardware.
- FSMN conv (11 taps, delta -9..+1): the center tap (delta 0, incl. the +1
  identity) is folded into the shrink weights via a per-channel rescale, so
  the shrink PSUM already holds the center term. The pure h' is copied to
  SBUF (Act, 1024-wide), then the residual identity (l>0) and the PE-assigned
  taps accumulate as blockdiag-diagonal matmuls INTO the same shrink PSUM
  (start=False). The DVE absorbs that PSUM in its first scalar_tensor_tensor
  FMA; remaining taps chain on DVE then Pool (SBUF-only engine).
- Expand relu+bias on Act (1024-wide PSUM reads).
"""

import sys

sys.path.insert(0, "/opt/trn_rl_repo")
from contextlib import ExitStack

import numpy as np

import concourse.bass as bass  # noqa: F401
import concourse.mybir as mybir
import concourse.tile as tile
from concourse import bacc
from concourse.bass_utils import run_bass_kernel_spmd


F32 = mybir.dt.float32
BF16 = mybir.dt.bfloat16
AF = mybir.ActivationFunctionType
OP = mybir.AluOpType

NCORES = 8
B, T, C, F = 32, 2048, 4, 120
DL, DP, L, LO, RO, S = 128, 64, 5, 10, 1, 5
BPC = B // NCORES  # batches per core
SEQ = BPC * C  # sequences per core
H = T // 2  # half-sequence length (halves stacked on partitions)
HALO_L = LO - 1  # 9 left halo columns
HW = HALO_L + H + RO  # h buffer width: 1034
NW = T // 512  # 512-token matmul windows per sequence

# conv tap deltas (excluding folded center 0): -9..-1 and +1.
# PE taps run as diagonal matmuls into the shrink PSUM. The Pool engine only
# supports tensor_tensor/tensor_scalar/copy (no FMA, no PSUM), so the
# remaining taps run on the DVE: the first as an stt FMA absorbing the shrink
# PSUM, the rest as tensor_scalar_mul (4x bf16) + tensor_tensor add (2x).
TAPS_PE = [-9, -8, -7, -6, -5, -4]
TAP_DVE_ABSORB = +1
TAPS_DVE_MULADD = [-3, -2]
TAPS_ACT_MUL = [-1]
NPE = len(TAPS_PE)


def build_nc():
    nc = bacc.Bacc("TRN2", target_bir_lowering=False, debug=False, num_devices=NCORES)

    xt_d = nc.dram_tensor("xt", [SEQ, F, T], BF16, kind="ExternalInput")
    we0_d = nc.dram_tensor("we0", [F, DL], BF16, kind="ExternalInput")
    # expand weights for l=1..4 and final, rows duplicated for both halves
    wed_d = nc.dram_tensor("wed", [2 * DP, 5 * DL], BF16, kind="ExternalInput")
    # shrink weights, center-tap folded (per-out-channel scale)
    wsp_d = nc.dram_tensor("wsp", [DL, L * DP], BF16, kind="ExternalInput")
    # blockdiag diagonal tap matrices for PE taps: [128, L*NPE*128]
    diag_d = nc.dram_tensor("diag", [2 * DP, L * NPE * 2 * DP], BF16, kind="ExternalInput")
    ident_d = nc.dram_tensor("ident", [2 * DP, 2 * DP], BF16, kind="ExternalInput")
    # rescaled taps (stt scalars), col = l*11 + (delta+9)
    taps_d = nc.dram_tensor("taps", [2 * DP, L * 11], F32, kind="ExternalInput")
    biases_d = nc.dram_tensor("biases", [DL, L + 1], F32, kind="ExternalInput")
    wd_d = nc.dram_tensor("wd", [DL, S], BF16, kind="ExternalInput")
    bd_d = nc.dram_tensor("bd", [S, 1], F32, kind="ExternalInput")
    out_d = nc.dram_tensor("out", [BPC, S, T], F32, kind="ExternalOutput")

    with tile.TileContext(nc) as tc, ExitStack() as ctx:
        wp = ctx.enter_context(tc.tile_pool(name="weights", bufs=1))
        xp = ctx.enter_context(tc.tile_pool(name="x", bufs=5))
        ep = ctx.enter_context(tc.tile_pool(name="e", bufs=3))
        hp = ctx.enter_context(tc.tile_pool(name="h", bufs=3))
        op_ = ctx.enter_context(tc.tile_pool(name="o", bufs=20))
        fp = ctx.enter_context(tc.tile_pool(name="f", bufs=6))
        pp = ctx.enter_context(tc.tile_pool(name="pooled", bufs=2))
        osb = ctx.enter_context(tc.tile_pool(name="osb", bufs=2))
        ps = ctx.enter_context(tc.tile_pool(name="ps", bufs=2, space="PSUM"))
        psh = ctx.enter_context(tc.tile_pool(name="psh", bufs=2, space="PSUM"))

        # --- weights / constants (loaded once) ---
        we0_sb = wp.tile([F, DL], BF16)
        nc.sync.dma_start(out=we0_sb[:], in_=we0_d[:])
        wed_sb = wp.tile([2 * DP, 5 * DL], BF16)
        nc.sync.dma_start(out=wed_sb[:], in_=wed_d[:])
        wsp_sb = wp.tile([DL, L * DP], BF16)
        nc.sync.dma_start(out=wsp_sb[:], in_=wsp_d[:])
        diag_sb = wp.tile([2 * DP, L * NPE * 2 * DP], BF16)
        nc.sync.dma_start(out=diag_sb[:], in_=diag_d[:])
        ident_sb = wp.tile([2 * DP, 2 * DP], BF16)
        nc.sync.dma_start(out=ident_sb[:], in_=ident_d[:])
        taps_sb = wp.tile([2 * DP, L * 11], F32)
        nc.sync.dma_start(out=taps_sb[:], in_=taps_d[:])
        bias_sb = wp.tile([DL, L + 1], F32)
        nc.sync.dma_start(out=bias_sb[:], in_=biases_d[:])
        wd_sb = wp.tile([DL, S], BF16)
        nc.sync.dma_start(out=wd_sb[:], in_=wd_d[:])
        bd_sb = wp.tile([S, 1], F32)
        nc.sync.dma_start(out=bd_sb[:], in_=bd_d[:])

        def tap(l, d):
            j = d + 9
            return taps_sb[:, l * 11 + j : l * 11 + j + 1]

        def diag(l, s):
            col = (l * NPE + s) * 2 * DP
            return diag_sb[:, col : col + 2 * DP]

        o_tiles = [None] * SEQ
        x_tiles = [None] * SEQ

        for s in range(min(5, SEQ)):
            x_tiles[s] = xp.tile([F, T], BF16, name="x_sb")
            nc.sync.dma_start(out=x_tiles[s][:], in_=xt_d[s])

        def expand(l, s, dst_sb, lcol, bias_col):
            # dst_sb [128, 2048] = relu(o_prev @ We + be); row-tile pairs
            o_prev = o_tiles[s]
            pes = [ps.tile([DL, 1024], F32, tag="ps", name="pe2") for _ in range(2)]
            for w in range(2):
                for half in range(2):
                    q = half * DP
                    nc.tensor.matmul(
                        pes[half][:, w * 512 : (w + 1) * 512],
                        wed_sb[q : q + DP, lcol : lcol + DL],
                        o_prev[q : q + DP, w * 512 : (w + 1) * 512],
                        tile_position=(q, 0),
                    )
            for half in range(2):
                nc.scalar.activation(
                    dst_sb[:, half * 1024 : (half + 1) * 1024],
                    pes[half][:],
                    AF.Relu,
                    bias=bias_sb[:, bias_col : bias_col + 1],
                    scale=1.0,
                )

        tmp_pool = ctx.enter_context(tc.tile_pool(name="tmp", bufs=8))

        def conv_stage(l, s):
            # residual + PE taps accumulate into psh(s) (start=False),
            # weight-major so walrus can dedup consecutive ldweights;
            # then the DVE absorbs psh and chains the remaining taps.
            ph, h_sb = pend[s]
            for si, d in enumerate(TAPS_PE):
                last = si == NPE - 1
                for w in range(2):
                    a = HALO_L + d + w * 512
                    nc.tensor.matmul(
                        ph[:, w * 512 : (w + 1) * 512],
                        diag(l, si),
                        h_sb[:, a : a + 512],
                        start=False,
                        stop=last,
                        skip_group_check=True,
                    )

            o_new = op_.tile([2 * DP, H], BF16, name="o_new")
            d0 = TAP_DVE_ABSORB
            nc.vector.scalar_tensor_tensor(
                o_new[:],
                h_sb[:, HALO_L + d0 : HALO_L + d0 + H],
                tap(l, d0),
                ph[:],
                OP.mult,
                OP.add,
            )
            for d in TAPS_DVE_MULADD:
                a = HALO_L + d
                tmp = tmp_pool.tile([2 * DP, H], BF16, name="tmp_sb")
                nc.vector.tensor_scalar_mul(tmp[:], h_sb[:, a : a + H], tap(l, d))
                nc.vector.tensor_add(o_new[:], o_new[:], tmp[:])
            if l > 0:
                # residual add on the DVE (cheaper than an identity matmul
                # occupying the PE, which is the bottleneck engine)
                nc.vector.tensor_add(o_new[:], o_new[:], o_tiles[s][:])
            for d in TAPS_ACT_MUL:
                # product on the Act engine, final add on DVE
                a = HALO_L + d
                tmp = tmp_pool.tile([2 * DP, H], BF16, name="tmp_sb")
                nc.scalar.activation(
                    tmp[:], h_sb[:, a : a + H], AF.Identity,
                    bias=0.0, scale=tap(l, d),
                )
                nc.vector.tensor_add(o_new[:], o_new[:], tmp[:])
            pend[s] = None
            o_tiles[s] = o_new

        pend = [None] * SEQ

        # h buffers as a manual ring: the pure-zero halo corners ([0:64,0:9]
        # left of half A, [64:128,1033] right of half B) are written by
        # nothing else, so zero them once instead of per sequence
        h_ring = []
        for hi in range(3):
            hbuf = hp.tile([2 * DP, HW], BF16, name="h_sb")
            nc.gpsimd.memset(hbuf[0:DP, 0:HALO_L], 0.0)
            nc.gpsimd.memset(hbuf[DP : 2 * DP, HALO_L + H : HW], 0.0)
            h_ring.append(hbuf)

        for l in range(L):
            for s in range(SEQ):
                # ---- expand ----
                e_sb = ep.tile([DL, T], BF16)
                if l == 0:
                    x_sb = x_tiles[s]
                    pes = [
                        ps.tile([DL, 1024], F32, tag="ps", name="pe2")
                        for _ in range(2)
                    ]
                    for w in range(NW):
                        nc.tensor.matmul(
                            pes[w // 2][:, (w % 2) * 512 : (w % 2 + 1) * 512],
                            we0_sb[:],
                            x_sb[:, w * 512 : (w + 1) * 512],
                        )
                    for half in range(2):
                        nc.scalar.activation(
                            e_sb[:, half * 1024 : (half + 1) * 1024],
                            pes[half][:],
                            AF.Relu,
                            bias=bias_sb[:, 0:1],
                            scale=1.0,
                        )
                    x_tiles[s] = None
                    if s + 5 < SEQ:
                        x_tiles[s + 5] = xp.tile([F, T], BF16, name="x_sb")
                        nc.sync.dma_start(out=x_tiles[s + 5][:], in_=xt_d[s + 5])
                else:
                    expand(l, s, e_sb, (l - 1) * DL, l)

                # ---- conv stage for the previous sequence (PE stays busy
                # while the Act engine runs this sequence's relu + h-copy) ----
                if s > 0:
                    conv_stage(l, s - 1)

                # ---- shrink (center-folded) into psh; col-tile pairs ----
                h_sb = h_ring[(l * SEQ + s) % len(h_ring)]
                ws_l = wsp_sb[:, l * DP : (l + 1) * DP]
                ph = psh.tile([2 * DP, 1024], F32, tag="ph")
                for w in range(2):
                    nc.tensor.matmul(
                        ph[0:DP, w * 512 : (w + 1) * 512],
                        ws_l,
                        e_sb[:, w * 512 : (w + 1) * 512],
                        skip_group_check=True,
                    )
                    nc.tensor.matmul(
                        ph[DP : 2 * DP, w * 512 : (w + 1) * 512],
                        ws_l,
                        e_sb[:, H + w * 512 : H + (w + 1) * 512],
                        skip_group_check=True,
                    )

                # ---- copy pure h' out, fill halos ----
                nc.scalar.copy(h_sb[:, HALO_L : HALO_L + H], ph[:])
                nc.gpsimd.tensor_copy(
                    h_sb[DP : 2 * DP, 0:HALO_L],
                    h_sb[0:DP, H : HALO_L + H],
                )
                nc.gpsimd.tensor_copy(
                    h_sb[0:DP, HALO_L + H : HW],
                    h_sb[DP : 2 * DP, HALO_L : HALO_L + RO],
                )
                pend[s] = (ph, h_sb)

            conv_stage(l, SEQ - 1)

        # ---- final expand + channel maxpool + decoder, per batch ----
        for b in range(BPC):
            f_tiles = []
            for c in range(C):
                s = b * C + c
                f_sb = fp.tile([DL, T], BF16)
                expand(L, s, f_sb, 4 * DL, L)
                f_tiles.append(f_sb)

            pooled = pp.tile([DL, T], BF16)
            nc.vector.tensor_max(pooled[:], f_tiles[0][:], f_tiles[1][:])
            nc.vector.tensor_max(pooled[:], pooled[:], f_tiles[2][:])
            nc.vector.tensor_max(pooled[:], pooled[:], f_tiles[3][:])

            out_sb = osb.tile([S, T], F32)
            for half in range(2):
                pd = ps.tile([S, 1024], F32, tag="ps", name="pe2")
                for w in range(2):
                    nc.tensor.matmul(
                        pd[:, w * 512 : (w + 1) * 512],
                        wd_sb[:],
                        pooled[:, half * 1024 + w * 512 : half * 1024 + (w + 1) * 512],
                    )
                nc.scalar.activation(
                    out_sb[:, half * 1024 : (half + 1) * 1024],
                    pd[:],
                    AF.Identity,
                    bias=bd_sb[:, 0:1],
                    scale=1.0,
                )
            nc.sync.dma_start(out=out_d[b], in_=out_sb[:])

    nc.compile()
    return nc


_NC = None


def get_nc():
    global _NC
    if _NC is None:
        _NC = build_nc()
    return _NC


def prep_in_maps(x, We0, be0, Ws0, wl0, wr0, We, be, Ws, wl, wr, We2, be2, Wd, bd):
    import ml_dtypes

    bf16 = ml_dtypes.bfloat16
    xt = np.ascontiguousarray(x.transpose(0, 2, 3, 1), dtype=np.float32)  # [B,C,F,T]

    # center-tap fold: t9c = 1 + wl[9]; shrink weights scaled by t9c per out
    # channel, other taps divided by t9c
    wl_full = np.concatenate([wl0[None], wl], axis=0)  # [L, 10, 64]
    wr_full = np.concatenate([wr0[None], wr], axis=0)  # [L, 1, 64]
    taps64 = np.concatenate([wl_full, wr_full], axis=1).copy()  # [L, 11, 64]
    t9c = 1.0 + taps64[:, LO - 1, :]  # [L, 64]
    taps64 = taps64 / t9c[:, None, :]  # rescaled; col 9 == 1 (folded)

    ws_all = np.stack([Ws0, Ws[0], Ws[1], Ws[2], Ws[3]])  # [L, 128, 64]
    ws_scaled = ws_all * t9c[:, None, :]  # fold center into shrink
    wsp = np.concatenate([ws_scaled[l] for l in range(L)], axis=1)  # [128, L*64]

    wed = np.concatenate(
        [np.concatenate([w, w], axis=0) for w in [We[0], We[1], We[2], We[3], We2]],
        axis=1,
    )  # [128, 5*128]

    biases = np.stack([be0, be[0], be[1], be[2], be[3], be2], axis=1)  # [128, 6]

    taps_tbl = np.tile(
        taps64.transpose(2, 0, 1).reshape(DP, L * 11), (2, 1)
    ).astype(np.float32)

    diag = np.zeros((L, NPE, 2 * DP, 2 * DP), np.float32)
    for l in range(L):
        for si, d in enumerate(TAPS_PE):
            np.fill_diagonal(diag[l, si], np.tile(taps64[l, d + 9, :], 2))
    diag2 = diag.transpose(2, 0, 1, 3).reshape(2 * DP, L * NPE * 2 * DP)
    ident = np.eye(2 * DP, dtype=np.float32)

    shared = dict(
        we0=np.ascontiguousarray(We0).astype(bf16),
        wed=np.ascontiguousarray(wed).astype(bf16),
        wsp=np.ascontiguousarray(wsp).astype(bf16),
        diag=np.ascontiguousarray(diag2).astype(bf16),
        ident=ident.astype(bf16),
        taps=np.ascontiguousarray(taps_tbl),
        biases=np.ascontiguousarray(biases.astype(np.float32)),
        wd=np.ascontiguousarray(Wd).astype(bf16),
        bd=np.ascontiguousarray(bd.reshape(S, 1), dtype=np.float32),
    )
    in_maps = []
    for k in range(NCORES):
        xs = xt[k * BPC : (k + 1) * BPC].reshape(SEQ, F, T)
        m = dict(shared)
        m["xt"] = np.ascontiguousarray(xs).astype(bf16)
        in_maps.append(m)
    return in_maps


def postprocess(results):
    full = np.concatenate([r["out"] for r in results], axis=0)  # [B, S, T]
    return np.ascontiguousarray(full.transpose(0, 2, 1))  # [B, T, S]


def kernel(**inputs):
    nc = get_nc()
    in_maps = prep_in_maps(**inputs)
    res = run_bass_kernel_spmd(nc, in_maps, core_ids=list(range(NCORES)))
    return postprocess(res.results)


# revision 16
# speedup vs baseline: 1.1934x; 1.1934x over previous
"""Trainium2 Bass kernel for nn_FSMNSeleNetV3 (FSMN stack + channel maxpool + decoder).

Self-contained: hardcodes all shapes from the problem spec and only imports
numpy + the concourse stack from /opt/trn_rl_repo.

Sharding: pure data parallel over batch. Each of the 8 cores processes 4
batches x 4 channels = 16 independent sequences of T=2048 tokens.

v3 design (layer-major, multi-engine balanced, PE tile-concurrency):
- All activations bf16 in SBUF; matmuls bf16 (1 cycle/row on the PE).
- 64-channel tensors (h, o) pack the two T/2 time-halves onto 128 partitions.
- Layer-major emission: each unit stage streams all 16 sequences so every
  engine always has independent work and the PE never idles (HAM stays warm).
- Expands (K=64) emit as adjacent row-group tile pairs (tile_position (0,0)
  and (64,0)); shrinks (M=64) as adjacent col-group pairs (out partition 0/64)
  - disjoint PE sub-arrays run these pairs concurrently on hardware.
- FSMN conv (11 taps, delta -9..+1): the center tap (delta 0, incl. the +1
  identity) is folded into the shrink weights via a per-channel rescale, so
  the shrink PSUM already holds the center term. The pure h' is copied to
  SBUF (Act, 1024-wide), then the residual identity (l>0) and the PE-assigned
  taps accumulate as blockdiag-diagonal matmuls INTO the same shrink PSUM
  (start=False). The DVE absorbs that PSUM in its first scalar_tensor_tensor
  FMA; remaining taps chain on DVE then Pool (SBUF-only engine).
- Expand relu+bias on Act (1024-wide PSUM reads).
"""

import sys

sys.path.insert(0, "/opt/trn_rl_repo")
from contextlib import ExitStack

import numpy as np

import concourse.bass as bass  # noqa: F401
import concourse.mybir as mybir
import concourse.tile as tile
from concourse import bacc
from concourse.bass_utils import run_bass_kernel_spmd


F32 = mybir.dt.float32
BF16 = mybir.dt.bfloat16
AF = mybir.ActivationFunctionType
OP = mybir.AluOpType

NCORES = 8
B, T, C, F = 32, 2048, 4, 120
DL, DP, L, LO, RO, S = 128, 64, 5, 10, 1, 5
BPC = B // NCORES  # batches per core
SEQ = BPC * C  # sequences per core
H = T // 2  # half-sequence length (halves stacked on partitions)
HALO_L = LO - 1  # 9 left halo columns
HW = HALO_L + H + RO  # h buffer width: 1034
NW = T // 512  # 512-token matmul windows per sequence

# conv tap deltas (excluding folded center 0): -9..-1 and +1.
# PE taps run as diagonal matmuls into the shrink PSUM. The Pool engine only
# supports tensor_tensor/tensor_scalar/copy (no FMA, no PSUM), so the
# remaining taps run on the DVE: the first as an stt FMA absorbing the shrink
# PSUM, the rest as tensor_scalar_mul (4x bf16) + tensor_tensor add (2x).
TAPS_PE = [-9, -8, -7, -6, -5, -4]
TAP_DVE_ABSORB = +1
TAPS_DVE_MULADD = [-3, -2]
TAPS_ACT_MUL = [-1]
NPE = len(TAPS_PE)


def build_nc():
    nc = bacc.Bacc("TRN2", target_bir_lowering=False, debug=False, num_devices=NCORES)

    xt_d = nc.dram_tensor("xt", [SEQ, F, T], BF16, kind="ExternalInput")
    we0_d = nc.dram_tensor("we0", [F, DL], BF16, kind="ExternalInput")
    # expand weights for l=1..4 and final, rows duplicated for both halves
    wed_d = nc.dram_tensor("wed", [2 * DP, 5 * DL], BF16, kind="ExternalInput")
    # shrink weights, center-tap folded (per-out-channel scale)
    wsp_d = nc.dram_tensor("wsp", [DL, L * DP], BF16, kind="ExternalInput")
    # blockdiag diagonal tap matrices for PE taps: [128, L*NPE*128]
    diag_d = nc.dram_tensor("diag", [2 * DP, L * NPE * 2 * DP], BF16, kind="ExternalInput")
    ident_d = nc.dram_tensor("ident", [2 * DP, 2 * DP], BF16, kind="ExternalInput")
    # rescaled taps (stt scalars), col = l*11 + (delta+9)
    taps_d = nc.dram_tensor("taps", [2 * DP, L * 11], F32, kind="ExternalInput")
    biases_d = nc.dram_tensor("biases", [DL, L + 1], F32, kind="ExternalInput")
    wd_d = nc.dram_tensor("wd", [DL, S], BF16, kind="ExternalInput")
    bd_d = nc.dram_tensor("bd", [S, 1], F32, kind="ExternalInput")
    out_d = nc.dram_tensor("out", [BPC, S, T], F32, kind="ExternalOutput")

    with tile.TileContext(nc) as tc, ExitStack() as ctx:
        wp = ctx.enter_context(tc.tile_pool(name="weights", bufs=1))
        xp = ctx.enter_context(tc.tile_pool(name="x", bufs=5))
        ep = ctx.enter_context(tc.tile_pool(name="e", bufs=3))
        hp = ctx.enter_context(tc.tile_pool(name="h", bufs=3))
        op_ = ctx.enter_context(tc.tile_pool(name="o", bufs=20))
        fp = ctx.enter_context(tc.tile_pool(name="f", bufs=6))
        pp = ctx.enter_context(tc.tile_pool(name="pooled", bufs=2))
        osb = ctx.enter_context(tc.tile_pool(name="osb", bufs=2))
        ps = ctx.enter_context(tc.tile_pool(name="ps", bufs=2, space="PSUM"))
        psh = ctx.enter_context(tc.tile_pool(name="psh", bufs=2, space="PSUM"))

        # --- weights / constants (loaded once) ---
        we0_sb = wp.tile([F, DL], BF16)
        nc.sync.dma_start(out=we0_sb[:], in_=we0_d[:])
        wed_sb = wp.tile([2 * DP, 5 * DL], BF16)
        nc.sync.dma_start(out=wed_sb[:], in_=wed_d[:])
        wsp_sb = wp.tile([DL, L * DP], BF16)
        nc.sync.dma_start(out=wsp_sb[:], in_=wsp_d[:])
        diag_sb = wp.tile([2 * DP, L * NPE * 2 * DP], BF16)
        nc.sync.dma_start(out=diag_sb[:], in_=diag_d[:])
        ident_sb = wp.tile([2 * DP, 2 * DP], BF16)
        nc.sync.dma_start(out=ident_sb[:], in_=ident_d[:])
        taps_sb = wp.tile([2 * DP, L * 11], F32)
        nc.sync.dma_start(out=taps_sb[:], in_=taps_d[:])
        bias_sb = wp.tile([DL, L + 1], F32)
        nc.sync.dma_start(out=bias_sb[:], in_=biases_d[:])
        wd_sb = wp.tile([DL, S], BF16)
        nc.sync.dma_start(out=wd_sb[:], in_=wd_d[:])
        bd_sb = wp.tile([S, 1], F32)
        nc.sync.dma_start(out=bd_sb[:], in_=bd_d[:])

        def tap(l, d):
            j = d + 9
            return taps_sb[:, l * 11 + j : l * 11 + j + 1]

        def diag(l, s):
            col = (l * NPE + s) * 2 * DP
            return diag_sb[:, col : col + 2 * DP]

        o_tiles = [None] * SEQ
        x_tiles = [None] * SEQ

        for s in range(min(5, SEQ)):
            x_tiles[s] = xp.tile([F, T], BF16, name="x_sb")
            nc.sync.dma_start(out=x_tiles[s][:], in_=xt_d[s])

        def expand(l, s, dst_sb, lcol, bias_col):
            # dst_sb [128, 2048] = relu(o_prev @ We + be); row-tile pairs
            o_prev = o_tiles[s]
            pes = [ps.tile([DL, 1024], F32, tag="ps", name="pe2") for _ in range(2)]
            for w in range(2):
                for half in range(2):
                    q = half * DP
                    nc.tensor.matmul(
                        pes[half][:, w * 512 : (w + 1) * 512],
                        wed_sb[q : q + DP, lcol : lcol + DL],
                        o_prev[q : q + DP, w * 512 : (w + 1) * 512],
                        tile_position=(q, 0),
                    )
            for half in range(2):
                nc.scalar.activation(
                    dst_sb[:, half * 1024 : (half + 1) * 1024],
                    pes[half][:],
                    AF.Relu,
                    bias=bias_sb[:, bias_col : bias_col + 1],
                    scale=1.0,
                )

        tmp_pool = ctx.enter_context(tc.tile_pool(name="tmp", bufs=8))

        def conv_stage(l, s):
            # residual + PE taps accumulate into psh(s) (start=False),
            # weight-major so walrus can dedup consecutive ldweights;
            # then the DVE absorbs psh and chains the remaining taps.
            ph, h_sb = pend[s]
            if l > 0:
                o_prev = o_tiles[s]
                for w in range(2):
                    nc.tensor.matmul(
                        ph[:, w * 512 : (w + 1) * 512],
                        ident_sb[:],
                        o_prev[:, w * 512 : (w + 1) * 512],
                        start=False,
                        stop=False,
                        skip_group_check=True,
                    )
            for si, d in enumerate(TAPS_PE):
                last = si == NPE - 1
                for w in range(2):
                    a = HALO_L + d + w * 512
                    nc.tensor.matmul(
                        ph[:, w * 512 : (w + 1) * 512],
                        diag(l, si),
                        h_sb[:, a : a + 512],
                        start=False,
                        stop=last,
                        skip_group_check=True,
                    )

            o_new = op_.tile([2 * DP, H], BF16, name="o_new")
            d0 = TAP_DVE_ABSORB
            nc.vector.scalar_tensor_tensor(
                o_new[:],
                h_sb[:, HALO_L + d0 : HALO_L + d0 + H],
                tap(l, d0),
                ph[:],
                OP.mult,
                OP.add,
            )
            for d in TAPS_DVE_MULADD:
                a = HALO_L + d
                tmp = tmp_pool.tile([2 * DP, H], BF16, name="tmp_sb")
                nc.vector.tensor_scalar_mul(tmp[:], h_sb[:, a : a + H], tap(l, d))
                nc.vector.tensor_add(o_new[:], o_new[:], tmp[:])
            for d in TAPS_ACT_MUL:
                # product on the Act engine, final add on DVE
                a = HALO_L + d
                tmp = tmp_pool.tile([2 * DP, H], BF16, name="tmp_sb")
                nc.scalar.activation(
                    tmp[:], h_sb[:, a : a + H], AF.Identity,
                    bias=0.0, scale=tap(l, d),
                )
                nc.vector.tensor_add(o_new[:], o_new[:], tmp[:])
            pend[s] = None
            o_tiles[s] = o_new

        pend = [None] * SEQ

        # h buffers as a manual ring: the pure-zero halo corners ([0:64,0:9]
        # left of half A, [64:128,1033] right of half B) are written by
        # nothing else, so zero them once instead of per sequence
        h_ring = []
        for hi in range(3):
            hbuf = hp.tile([2 * DP, HW], BF16, name="h_sb")
            nc.gpsimd.memset(hbuf[0:DP, 0:HALO_L], 0.0)
            nc.gpsimd.memset(hbuf[DP : 2 * DP, HALO_L + H : HW], 0.0)
            h_ring.append(hbuf)

        for l in range(L):
            for s in range(SEQ):
                # ---- expand ----
                e_sb = ep.tile([DL, T], BF16)
                if l == 0:
                    x_sb = x_tiles[s]
                    pes = [
                        ps.tile([DL, 1024], F32, tag="ps", name="pe2")
                        for _ in range(2)
                    ]
                    for w in range(NW):
                        nc.tensor.matmul(
                            pes[w // 2][:, (w % 2) * 512 : (w % 2 + 1) * 512],
                            we0_sb[:],
                            x_sb[:, w * 512 : (w + 1) * 512],
                        )
                    for half in range(2):
                        nc.scalar.activation(
                            e_sb[:, half * 1024 : (half + 1) * 1024],
                            pes[half][:],
                            AF.Relu,
                            bias=bias_sb[:, 0:1],
                            scale=1.0,
                        )
                    x_tiles[s] = None
                    if s + 5 < SEQ:
                        x_tiles[s + 5] = xp.tile([F, T], BF16, name="x_sb")
                        nc.sync.dma_start(out=x_tiles[s + 5][:], in_=xt_d[s + 5])
                else:
                    expand(l, s, e_sb, (l - 1) * DL, l)

                # ---- conv stage for the previous sequence (PE stays busy
                # while the Act engine runs this sequence's relu + h-copy) ----
                if s > 0:
                    conv_stage(l, s - 1)

                # ---- shrink (center-folded) into psh; col-tile pairs ----
                h_sb = h_ring[(l * SEQ + s) % len(h_ring)]
                ws_l = wsp_sb[:, l * DP : (l + 1) * DP]
                ph = psh.tile([2 * DP, 1024], F32, tag="ph")
                for w in range(2):
                    nc.tensor.matmul(
                        ph[0:DP, w * 512 : (w + 1) * 512],
                        ws_l,
                        e_sb[:, w * 512 : (w + 1) * 512],
                        skip_group_check=True,
                    )
                    nc.tensor.matmul(
                        ph[DP : 2 * DP, w * 512 : (w + 1) * 512],
                        ws_l,
                        e_sb[:, H + w * 512 : H + (w + 1) * 512],
                        skip_group_check=True,
                    )

                # ---- copy pure h' out, fill halos ----
                nc.scalar.copy(h_sb[:, HALO_L : HALO_L + H], ph[:])
                nc.gpsimd.tensor_copy(
                    h_sb[DP : 2 * DP, 0:HALO_L],
                    h_sb[0:DP, H : HALO_L + H],
                )
                nc.gpsimd.tensor_copy(
                    h_sb[0:DP, HALO_L + H : HW],
                    h_sb[DP : 2 * DP, HALO_L : HALO_L + RO],
                )
                pend[s] = (ph, h_sb)

            conv_stage(l, SEQ - 1)

        # ---- final expand + channel maxpool + decoder, per batch ----
        for b in range(BPC):
            f_tiles = []
            for c in range(C):
                s = b * C + c
                f_sb = fp.tile([DL, T], BF16)
                expand(L, s, f_sb, 4 * DL, L)
                f_tiles.append(f_sb)

            pooled = pp.tile([DL, T], BF16)
            nc.vector.tensor_max(pooled[:], f_tiles[0][:], f_tiles[1][:])
            nc.vector.tensor_max(pooled[:], pooled[:], f_tiles[2][:])
            nc.vector.tensor_max(pooled[:], pooled[:], f_tiles[3][:])

            out_sb = osb.tile([S, T], F32)
            for half in range(2):
                pd = ps.tile([S, 1024], F32, tag="ps", name="pe2")
                for w in range(2):
                    nc.tensor.matmul(
                        pd[:, w * 512 : (w + 1) * 512],
                        wd_sb[:],
                        pooled[:, half * 1024 + w * 512 : half * 1024 + (w + 1) * 512],
                    )
                nc.scalar.activation(
                    out_sb[:, half * 1024 : (half + 1) * 1024],
                    pd[:],
                    AF.Identity,
                    bias=bd_sb[:, 0:1],
                    scale=1.0,
                )
            nc.sync.dma_start(out=out_d[b], in_=out_sb[:])

    nc.compile()
    return nc


_NC = None


def get_nc():
    global _NC
    if _NC is None:
        _NC = build_nc()
    return _NC


def prep_in_maps(x, We0, be0, Ws0, wl0, wr0, We, be, Ws, wl, wr, We2, be2, Wd, bd):
    import ml_dtypes

    bf16 = ml_dtypes.bfloat16
    xt = np.ascontiguousarray(x.transpose(0, 2, 3, 1), dtype=np.float32)  # [B,C,F,T]

    # center-tap fold: t9c = 1 + wl[9]; shrink weights scaled by t9c per out
    # channel, other taps divided by t9c
    wl_full = np.concatenate([wl0[None], wl], axis=0)  # [L, 10, 64]
    wr_full = np.concatenate([wr0[None], wr], axis=0)  # [L, 1, 64]
    taps64 = np.concatenate([wl_full, wr_full], axis=1).copy()  # [L, 11, 64]
    t9c = 1.0 + taps64[:, LO - 1, :]  # [L, 64]
    taps64 = taps64 / t9c[:, None, :]  # rescaled; col 9 == 1 (folded)

    ws_all = np.stack([Ws0, Ws[0], Ws[1], Ws[2], Ws[3]])  # [L, 128, 64]
    ws_scaled = ws_all * t9c[:, None, :]  # fold center into shrink
    wsp = np.concatenate([ws_scaled[l] for l in range(L)], axis=1)  # [128, L*64]

    wed = np.concatenate(
        [np.concatenate([w, w], axis=0) for w in [We[0], We[1], We[2], We[3], We2]],
        axis=1,
    )  # [128, 5*128]

    biases = np.stack([be0, be[0], be[1], be[2], be[3], be2], axis=1)  # [128, 6]

    taps_tbl = np.tile(
        taps64.transpose(2, 0, 1).reshape(DP, L * 11), (2, 1)
    ).astype(np.float32)

    diag = np.zeros((L, NPE, 2 * DP, 2 * DP), np.float32)
    for l in range(L):
        for si, d in enumerate(TAPS_PE):
            np.fill_diagonal(diag[l, si], np.tile(taps64[l, d + 9, :], 2))
    diag2 = diag.transpose(2, 0, 1, 3).reshape(2 * DP, L * NPE * 2 * DP)
    ident = np.eye(2 * DP, dtype=np.float32)

    shared = dict(
        we0=np.ascontiguousarray(We0).astype(bf16),
        wed=np.ascontiguousarray(wed).astype(bf16),
        wsp=np.ascontiguousarray(wsp).astype(bf16),
        diag=np.ascontiguousarray(diag2).astype(bf16),
        ident=ident.astype(bf16),
        taps=np.ascontiguousarray(taps_tbl),
        biases=np.ascontiguousarray(biases.astype(np.float32)),
        wd=np.ascontiguousarray(Wd).astype(bf16),
        bd=np.ascontiguousarray(bd.reshape(S, 1), dtype=np.float32),
    )
    in_maps = []
    for k in range(NCORES):
        xs = xt[k * BPC : (k + 1) * BPC].reshape(SEQ, F, T)
        m = dict(shared)
        m["xt"] = np.ascontiguousarray(xs).astype(bf16)
        in_maps.append(m)
    return in_maps


def postprocess(results):
    full = np.concatenate([r["out"] for r in results], axis=0)  # [B, S, T]
    return np.ascontiguousarray(full.transpose(0, 2, 1))  # [B, T, S]


def kernel(**inputs):
    nc = get_nc()
    in_maps = prep_in_maps(**inputs)
    res = run_bass_kernel_spmd(nc, in_maps, core_ids=list(range(NCORES)))
    return postprocess(res.results)
